# revision 12
# baseline (speedup 1.0000x reference)
"""Trainium2 Bass kernel for nn_CrossOp (cross conv: pairwise target/support 3x3 conv).

Problem (hardcoded): B=4, SX=1, SY=16, C=CO=64, K=3, H=W=128.
reference computes, for every (b, s) pair:
    interaction[b,s] = conv3x3(concat(target[b], support[b,s]), weight) + bias
    new_target[b]    = mean_s interaction[b,s]

Strategy:
  * conv splits over the channel concat: conv_t[b] (once per batch) + conv_s[b,s].
  * Shard rows of H across the 8 cores (16 rows each); every core convolves all
    68 images (64 support + 4 target) over its row slice, so no cross-core
    reduction is needed.
  * Images are processed two-at-a-time stacked on the 128 SBUF partitions with
    block-diagonal weights ([[Ws,0],[0,Ws]]) -> K=128, M=128 matmuls, one pass
    per conv tap, 9 taps + (for targets) a K=1 bias tap accumulated in PSUM.
    Support images pair across batches (b0 with b1, b2 with b3) so the
    conv_t+bias tile produced by the paired target conv lines up with both
    partition halves directly.
  * float32r matmuls: full PE rate at N=512, ~1e-4 relative error.
  * Host pre-pads inputs (1px zero halo in H and W) so the device program is
    completely uniform across cores.
  * new_target is the mean over the gathered interaction, computed on host.
"""

import os
import sys

import numpy as np

sys.path.insert(0, "/opt/trn_rl_repo")

import concourse.bass as bass
import concourse.bass_utils as bass_utils
import concourse.mybir as mybir
from concourse.bass_utils import run_bass_kernel_spmd
from concourse.tile import TileContext

TAP_OUTER = os.environ.get("KERNEL_TAP_OUTER", "0") == "1"

if os.environ.get("KERNEL_LDW_OPT", "0") == "1" and not getattr(
    bass_utils, "_ldw_opt_patched", False
):
    _orig_run_command = bass_utils.run_command

    def _run_command_ldw(argv, **kwargs):
        argv = [
            a.replace("--enable-ldw-opt=false", "--enable-ldw-opt=true") for a in argv
        ]
        return _orig_run_command(argv, **kwargs)

    bass_utils.run_command = _run_command_ldw
    bass_utils._ldw_opt_patched = True

B, SX, SY, C, CO, KK, H, W = 4, 1, 16, 64, 64, 3, 128, 128
NCORES = 8
RB = H // NCORES          # output rows per core = 16
HR = RB + 2               # halo'd rows per core = 18
WP = W + 2                # padded width = 130
NIMG = B * SY             # 64 support images
NPAIR = NIMG // 2         # 32 support pairs
PXT = 4                   # psum tiles per row-block (each 4 rows x 128 = 512)
NPX = RB * W              # 2048 output pixels per image slice
NWARM = 30                # PE warm-up matmuls (overlap the initial DMAs)

F32 = mybir.dt.float32
F32R = mybir.dt.float32r


def _build_program():
    nc = bass.Bass()
    sup_d = nc.dram_tensor("sup", [NIMG, C, HR * WP], F32R, kind="ExternalInput")
    tgt_d = nc.dram_tensor("tgt", [2, 128, HR * WP], F32R, kind="ExternalInput")
    wp_d = nc.dram_tensor("wp", [128, 9 * 128], F32R, kind="ExternalInput")
    wt_d = nc.dram_tensor("wt", [128, 9 * 128], F32R, kind="ExternalInput")
    bias_d = nc.dram_tensor("bias2", [1, 128], F32R, kind="ExternalInput")
    ones_d = nc.dram_tensor("ones", [1, 512], F32R, kind="ExternalInput")
    out_d = nc.dram_tensor("out", [NPAIR, 128, NPX], F32, kind="ExternalOutput")
    warm_d = nc.dram_tensor("warm", [128, 512], F32, kind="ExternalOutput")

    with TileContext(nc) as tc:
        with (
            tc.tile_pool(name="consts", bufs=1) as cpool,
            tc.tile_pool(name="cb", bufs=1) as cbpool,
            tc.tile_pool(name="inp", bufs=4) as ipool,
            tc.tile_pool(name="outp", bufs=3) as opool,
            tc.tile_pool(name="ps", bufs=8, space="PSUM") as pspool,
        ):
            wp_t = cpool.tile([128, 9 * 128], F32R, tag="wp")
            wt_t = cpool.tile([128, 9 * 128], F32R, tag="wt")
            bias_t = cpool.tile([1, 128], F32R, tag="bias")
            ones_t = cpool.tile([1, 512], F32R, tag="ones")
            tgt_t = cpool.tile([128, 2 * HR * WP], F32R, tag="tgt")
            nc.sync.dma_start(out=wp_t[:], in_=wp_d[:])
            nc.sync.dma_start(out=wt_t[:], in_=wt_d[:])
            nc.sync.dma_start(out=bias_t[:], in_=bias_d[:])
            nc.sync.dma_start(out=ones_t[:], in_=ones_d[:])
            nc.sync.dma_start(
                out=tgt_t.rearrange("p (a n) -> p a n", a=2),
                in_=tgt_d.rearrange("a p n -> p a n"),
            )

            # PE warm-up: dense K=128/M=128/N=512 matmuls on the weight tile
            # (first DMA to land), overlapping the other input DMAs, so the
            # HAM clock-gate opens before the real convolution stream starts.
            wps = pspool.tile([128, 512], F32, tag="ps")
            for i in range(NWARM):
                nc.tensor.matmul(
                    wps[:],
                    wp_t[:, :128],
                    wp_t[:, :512],
                    start=(i == 0),
                    stop=(i == NWARM - 1),
                )
            warm_t = cpool.tile([128, 512], F32, tag="warm")
            nc.vector.tensor_copy(out=warm_t[:], in_=wps[:])
            nc.sync.dma_start(out=warm_d[:], in_=warm_t[:])

            tgt_v = tgt_t.rearrange("p (a r w) -> p a r w", a=2, w=WP)

            # Stage 1: paired target convs ([ct_2q; ct_2q+1]) + bias tap,
            # copied to SBUF via ACT -> cb tiles.
            cb_t = cbpool.tile([128, 2 * NPX], F32, tag="cb")
            cb_v = cb_t.rearrange("p (q n) -> p q n", q=2)
            for q in range(2):
                for px in range(PXT):
                    ps = pspool.tile([128, 512], F32, tag="ps")
                    r0 = px * PXT
                    for t in range(9):
                        ky, kx = divmod(t, 3)
                        rhs = tgt_v[:, q, r0 + ky : r0 + ky + 4, kx : kx + W]
                        nc.tensor.matmul(
                            ps[:],
                            wt_t[:, t * 128 : (t + 1) * 128],
                            rhs,
                            start=(t == 0),
                            stop=False,
                        )
                    # bias as a K=1 tap: ones[1,512] x bias2[1,128]
                    nc.tensor.matmul(
                        ps[:], bias_t[:], ones_t[:], start=False, stop=True
                    )
                    nc.scalar.activation(
                        cb_v[:, q, px * 512 : (px + 1) * 512],
                        ps[:],
                        mybir.ActivationFunctionType.Copy,
                    )

            # Stage 2: support pairs; 9-tap accumulation with block-diagonal
            # weights, then interaction = psum + cb via DVE, DMA out.
            for p in range(NPAIR):
                q = p // 16
                in_t = ipool.tile([128, HR * WP], F32R, tag="in")
                nc.sync.dma_start(
                    out=in_t[:],
                    in_=sup_d[2 * p : 2 * p + 2].rearrange("a c n -> (a c) n"),
                )
                in_v = in_t.rearrange("p (r w) -> p r w", w=WP)
                out_t = opool.tile([128, NPX], F32, tag="out")
                if TAP_OUTER:
                    pss = [
                        pspool.tile([128, 512], F32, tag="ps", name=f"ps{p}_{j}")
                        for j in range(PXT)
                    ]
                    for t in range(9):
                        ky, kx = divmod(t, 3)
                        for px in range(PXT):
                            r0 = px * PXT
                            rhs = in_v[:, r0 + ky : r0 + ky + 4, kx : kx + W]
                            nc.tensor.matmul(
                                pss[px][:],
                                wp_t[:, t * 128 : (t + 1) * 128],
                                rhs,
                                start=(t == 0),
                                stop=(t == 8),
                            )
                    for px in range(PXT):
                        nc.vector.tensor_add(
                            out_t[:, px * 512 : (px + 1) * 512],
                            pss[px][:],
                            cb_v[:, q, px * 512 : (px + 1) * 512],
                        )
                else:
                    for px in range(PXT):
                        ps = pspool.tile([128, 512], F32, tag="ps")
                        r0 = px * PXT
                        for t in range(9):
                            ky, kx = divmod(t, 3)
                            rhs = in_v[:, r0 + ky : r0 + ky + 4, kx : kx + W]
                            nc.tensor.matmul(
                                ps[:],
                                wp_t[:, t * 128 : (t + 1) * 128],
                                rhs,
                                start=(t == 0),
                                stop=(t == 8),
                            )
                        nc.vector.tensor_add(
                            out_t[:, px * 512 : (px + 1) * 512],
                            ps[:],
                            cb_v[:, q, px * 512 : (px + 1) * 512],
                        )
                nc.sync.dma_start(out=out_d[p], in_=out_t[:])

    _split_excess_waits(nc)
    return nc


def _split_excess_waits(nc):
    """Walrus rejects >1 embedded sync-wait on several lowered instruction
    forms (fp32/f32r self-loading Matmult, Drain, NoOp). Move every embedded
    wait onto standalone per-wait EventSemaphore instructions on the same
    engine, immediately before the instruction."""
    for f in nc.m.functions:
        for blk in f.blocks:
            new = []
            for ins in blk.instructions:
                si = ins.sync_info
                if si is not None and len(si.on_wait) > 1:
                    for k, w_ in enumerate(si.on_wait):
                        new.append(
                            mybir.InstEventSemaphore(
                                name=f"{ins.name}-w{k}",
                                engine=ins.engine,
                                sync_info=mybir.SyncInfo(on_wait=[w_], on_update=[]),
                            )
                        )
                    ins.sync_info = mybir.SyncInfo(
                        on_wait=[], on_update=list(si.on_update)
                    )
                new.append(ins)
            blk.instructions = new


def _prep_inputs(target, support, weight, bias):
    target = np.asarray(target, dtype=np.float32)
    support = np.asarray(support, dtype=np.float32)
    weight = np.asarray(weight, dtype=np.float32)
    bias = np.asarray(bias, dtype=np.float32)

    supf = support.reshape(B, SY, C, H, W)
    tgtf = target.reshape(B, C, H, W)
    # pair support images across batch groups: pair p = q*16 + s holds
    # (img(2q, s), img(2q+1, s)) stacked on channels -> [NPAIR, 128, H, W]
    sup_pairs = (
        supf.reshape(2, 2, SY, C, H, W)
        .transpose(0, 2, 1, 3, 4, 5)
        .reshape(NPAIR, 2 * C, H, W)
    )
    sup_pad = np.zeros((NPAIR, 2 * C, H + 2, WP), np.float32)
    sup_pad[:, :, 1 : H + 1, 1 : W + 1] = sup_pairs
    tgt_pad = np.zeros((2, 2 * C, H + 2, WP), np.float32)
    tgt_pad[:, :, 1 : H + 1, 1 : W + 1] = tgtf.reshape(2, 2 * C, H, W)

    Wt = weight[:, :C]  # applies to target channels
    Ws = weight[:, C:]  # applies to support channels
    wp = np.zeros((128, 9, 128), np.float32)
    wt = np.zeros((128, 9, 128), np.float32)
    for t in range(9):
        ky, kx = divmod(t, 3)
        wp[:64, t, :64] = Ws[:, :, ky, kx].T
        wp[64:, t, 64:] = Ws[:, :, ky, kx].T
        wt[:64, t, :64] = Wt[:, :, ky, kx].T
        wt[64:, t, 64:] = Wt[:, :, ky, kx].T
    wp = np.ascontiguousarray(wp.reshape(128, 9 * 128))
    wt = np.ascontiguousarray(wt.reshape(128, 9 * 128))
    bias2 = np.concatenate([bias, bias]).reshape(1, 128).astype(np.float32)
    ones = np.ones((1, 512), np.float32)

    in_maps = []
    for i in range(NCORES):
        y0 = i * RB
        in_maps.append(
            {
                "sup": np.ascontiguousarray(sup_pad[:, :, y0 : y0 + HR, :]).reshape(
                    NIMG, C, HR * WP
                ),
                "tgt": np.ascontiguousarray(tgt_pad[:, :, y0 : y0 + HR, :]).reshape(
                    2, 128, HR * WP
                ),
                "wp": wp,
                "wt": wt,
                "bias2": bias2,
                "ones": ones,
            }
        )
    return in_maps


def _assemble(results):
    interaction = np.empty((B, SY, CO, H, W), np.float32)
    for i in range(NCORES):
        # out[p] = [2, CO, RB, W] with p = q*16+s covering batches (2q, 2q+1)
        o = results[i]["out"].reshape(2, SY, 2, CO, RB, W)
        o = o.transpose(0, 2, 1, 3, 4, 5).reshape(B, SY, CO, RB, W)
        interaction[:, :, :, i * RB : (i + 1) * RB, :] = o
    new_target = interaction.mean(axis=1, keepdims=True)
    return new_target, interaction


def run(target, support, weight, bias, trace=False, trace_kwargs=None):
    in_maps = _prep_inputs(target, support, weight, bias)
    nc = _build_program()
    kw = {}
    if trace:
        kw = dict(trace=True, **(trace_kwargs or {}))
    res = run_bass_kernel_spmd(nc, in_maps, core_ids=list(range(NCORES)), **kw)
    new_target, interaction = _assemble(res.results)
    return (new_target, interaction), res


def kernel(target, support, weight, bias):
    (new_target, interaction), _ = run(target, support, weight, bias)
    return new_target, interaction


# revision 13
# speedup vs baseline: 1.0090x; 1.0090x over previous
"""Trainium2 Bass kernel for nn_CrossOp (cross conv: pairwise target/support 3x3 conv).

Problem (hardcoded): B=4, SX=1, SY=16, C=CO=64, K=3, H=W=128.
reference computes, for every (b, s) pair:
    interaction[b,s] = conv3x3(concat(target[b], support[b,s]), weight) + bias
    new_target[b]    = mean_s interaction[b,s]

Strategy:
  * conv splits over the channel concat: conv_t[b] (once per batch) + conv_s[b,s].
  * Shard rows of H across the 8 cores (16 rows each); every core convolves all
    68 images (64 support + 4 target) over its row slice, so no cross-core
    reduction is needed.
  * Images are processed two-at-a-time stacked on the 128 SBUF partitions with
    block-diagonal weights ([[Ws,0],[0,Ws]]) -> K=128, M=128 matmuls, one pass
    per conv tap, 9 taps + (for targets) a K=1 bias tap accumulated in PSUM.
    Support images pair across batches (b0 with b1, b2 with b3) so the
    conv_t+bias tile produced by the paired target conv lines up with both
    partition halves directly.
  * float32r matmuls: full PE rate at N=512, ~1e-4 relative error.
  * Host pre-pads inputs (1px zero halo in H and W) so the device program is
    completely uniform across cores.
  * new_target is the mean over the gathered interaction, computed on host.
"""

import os
import sys

import numpy as np

sys.path.insert(0, "/opt/trn_rl_repo")

import concourse.bass as bass
import concourse.bass_utils as bass_utils
import concourse.mybir as mybir
from concourse.bass_utils import run_bass_kernel_spmd
from concourse.tile import TileContext

TAP_OUTER = os.environ.get("KERNEL_TAP_OUTER", "0") == "1"

if os.environ.get("KERNEL_LDW_OPT", "0") == "1" and not getattr(
    bass_utils, "_ldw_opt_patched", False
):
    _orig_run_command = bass_utils.run_command

    def _run_command_ldw(argv, **kwargs):
        argv = [
            a.replace("--enable-ldw-opt=false", "--enable-ldw-opt=true") for a in argv
        ]
        return _orig_run_command(argv, **kwargs)

    bass_utils.run_command = _run_command_ldw
    bass_utils._ldw_opt_patched = True

B, SX, SY, C, CO, KK, H, W = 4, 1, 16, 64, 64, 3, 128, 128
NCORES = 8
RB = H // NCORES          # output rows per core = 16
HR = RB + 2               # halo'd rows per core = 18
WP = W + 2                # padded width = 130
NIMG = B * SY             # 64 support images
NPAIR = NIMG // 2         # 32 support pairs
PXT = 4                   # psum tiles per row-block (each 4 rows x 128 = 512)
NPX = RB * W              # 2048 output pixels per image slice
NWARM = 18                # PE warm-up matmuls (overlap the initial DMAs)

F32 = mybir.dt.float32
F32R = mybir.dt.float32r


def _build_program():
    nc = bass.Bass()
    sup_d = nc.dram_tensor("sup", [NIMG, C, HR * WP], F32R, kind="ExternalInput")
    tgt_d = nc.dram_tensor("tgt", [2, 128, HR * WP], F32R, kind="ExternalInput")
    wp_d = nc.dram_tensor("wp", [128, 9 * 128], F32R, kind="ExternalInput")
    wt_d = nc.dram_tensor("wt", [128, 9 * 128], F32R, kind="ExternalInput")
    bias_d = nc.dram_tensor("bias2", [1, 128], F32R, kind="ExternalInput")
    ones_d = nc.dram_tensor("ones", [1, 512], F32R, kind="ExternalInput")
    out_d = nc.dram_tensor("out", [NPAIR, 128, NPX], F32, kind="ExternalOutput")
    warm_d = nc.dram_tensor("warm", [128, 512], F32, kind="ExternalOutput")

    with TileContext(nc) as tc:
        with (
            tc.tile_pool(name="consts", bufs=1) as cpool,
            tc.tile_pool(name="cb", bufs=1) as cbpool,
            tc.tile_pool(name="inp", bufs=4) as ipool,
            tc.tile_pool(name="outp", bufs=3) as opool,
            tc.tile_pool(name="ps", bufs=8, space="PSUM") as pspool,
        ):
            wp_t = cpool.tile([128, 9 * 128], F32R, tag="wp")
            wt_t = cpool.tile([128, 9 * 128], F32R, tag="wt")
            bias_t = cpool.tile([1, 128], F32R, tag="bias")
            ones_t = cpool.tile([1, 512], F32R, tag="ones")
            tgt_t = cpool.tile([128, 2 * HR * WP], F32R, tag="tgt")
            nc.sync.dma_start(out=wp_t[:], in_=wp_d[:])
            nc.sync.dma_start(out=wt_t[:], in_=wt_d[:])
            nc.sync.dma_start(out=bias_t[:], in_=bias_d[:])
            nc.sync.dma_start(out=ones_t[:], in_=ones_d[:])
            nc.sync.dma_start(
                out=tgt_t.rearrange("p (a n) -> p a n", a=2),
                in_=tgt_d.rearrange("a p n -> p a n"),
            )

            # PE warm-up: dense K=128/M=128/N=512 matmuls on the weight tile
            # (first DMA to land), overlapping the other input DMAs, so the
            # HAM clock-gate opens before the real convolution stream starts.
            wps = pspool.tile([128, 512], F32, tag="ps")
            for i in range(NWARM):
                nc.tensor.matmul(
                    wps[:],
                    wp_t[:, :128],
                    wp_t[:, :512],
                    start=(i == 0),
                    stop=(i == NWARM - 1),
                )
            warm_t = cpool.tile([128, 512], F32, tag="warm")
            nc.vector.tensor_copy(out=warm_t[:], in_=wps[:])
            nc.sync.dma_start(out=warm_d[:], in_=warm_t[:])

            tgt_v = tgt_t.rearrange("p (a r w) -> p a r w", a=2, w=WP)

            # Stage 1: paired target convs ([ct_2q; ct_2q+1]) + bias tap,
            # copied to SBUF via ACT -> cb tiles.
            cb_t = cbpool.tile([128, 2 * NPX], F32, tag="cb")
            cb_v = cb_t.rearrange("p (q n) -> p q n", q=2)
            for q in range(2):
                for px in range(PXT):
                    ps = pspool.tile([128, 512], F32, tag="ps")
                    r0 = px * PXT
                    for t in range(9):
                        ky, kx = divmod(t, 3)
                        rhs = tgt_v[:, q, r0 + ky : r0 + ky + 4, kx : kx + W]
                        nc.tensor.matmul(
                            ps[:],
                            wt_t[:, t * 128 : (t + 1) * 128],
                            rhs,
                            start=(t == 0),
                            stop=False,
                        )
                    # bias as a K=1 tap: ones[1,512] x bias2[1,128]
                    nc.tensor.matmul(
                        ps[:], bias_t[:], ones_t[:], start=False, stop=True
                    )
                    nc.scalar.activation(
                        cb_v[:, q, px * 512 : (px + 1) * 512],
                        ps[:],
                        mybir.ActivationFunctionType.Copy,
                    )

            # Stage 2: support pairs; 9-tap accumulation with block-diagonal
            # weights, then interaction = psum + cb via DVE, DMA out.
            for p in range(NPAIR):
                q = p // 16
                in_t = ipool.tile([128, HR * WP], F32R, tag="in")
                nc.sync.dma_start(
                    out=in_t[:],
                    in_=sup_d[2 * p : 2 * p + 2].rearrange("a c n -> (a c) n"),
                )
                in_v = in_t.rearrange("p (r w) -> p r w", w=WP)
                out_t = opool.tile([128, NPX], F32, tag="out")
                if TAP_OUTER:
                    pss = [
                        pspool.tile([128, 512], F32, tag="ps", name=f"ps{p}_{j}")
                        for j in range(PXT)
                    ]
                    for t in range(9):
                        ky, kx = divmod(t, 3)
                        for px in range(PXT):
                            r0 = px * PXT
                            rhs = in_v[:, r0 + ky : r0 + ky + 4, kx : kx + W]
                            nc.tensor.matmul(
                                pss[px][:],
                                wp_t[:, t * 128 : (t + 1) * 128],
                                rhs,
                                start=(t == 0),
                                stop=(t == 8),
                            )
                    for px in range(PXT):
                        nc.vector.tensor_add(
                            out_t[:, px * 512 : (px + 1) * 512],
                            pss[px][:],
                            cb_v[:, q, px * 512 : (px + 1) * 512],
                        )
                else:
                    for px in range(PXT):
                        ps = pspool.tile([128, 512], F32, tag="ps")
                        r0 = px * PXT
                        for t in range(9):
                            ky, kx = divmod(t, 3)
                            rhs = in_v[:, r0 + ky : r0 + ky + 4, kx : kx + W]
                            nc.tensor.matmul(
                                ps[:],
                                wp_t[:, t * 128 : (t + 1) * 128],
                                rhs,
                                start=(t == 0),
                                stop=(t == 8),
                            )
                        nc.vector.tensor_add(
                            out_t[:, px * 512 : (px + 1) * 512],
                            ps[:],
                            cb_v[:, q, px * 512 : (px + 1) * 512],
                        )
                nc.sync.dma_start(out=out_d[p], in_=out_t[:])

    _split_excess_waits(nc)
    return nc


def _split_excess_waits(nc):
    """Walrus rejects >1 embedded sync-wait on several lowered instruction
    forms (fp32/f32r self-loading Matmult, Drain, NoOp). Move every embedded
    wait onto standalone per-wait EventSemaphore instructions on the same
    engine, immediately before the instruction."""
    for f in nc.m.functions:
        for blk in f.blocks:
            new = []
            for ins in blk.instructions:
                si = ins.sync_info
                if si is not None and len(si.on_wait) > 1:
                    for k, w_ in enumerate(si.on_wait):
                        new.append(
                            mybir.InstEventSemaphore(
                                name=f"{ins.name}-w{k}",
                                engine=ins.engine,
                                sync_info=mybir.SyncInfo(on_wait=[w_], on_update=[]),
                            )
                        )
                    ins.sync_info = mybir.SyncInfo(
                        on_wait=[], on_update=list(si.on_update)
                    )
                new.append(ins)
            blk.instructions = new


def _prep_inputs(target, support, weight, bias):
    target = np.asarray(target, dtype=np.float32)
    support = np.asarray(support, dtype=np.float32)
    weight = np.asarray(weight, dtype=np.float32)
    bias = np.asarray(bias, dtype=np.float32)

    supf = support.reshape(B, SY, C, H, W)
    tgtf = target.reshape(B, C, H, W)
    # pair support images across batch groups: pair p = q*16 + s holds
    # (img(2q, s), img(2q+1, s)) stacked on channels -> [NPAIR, 128, H, W]
    sup_pairs = (
        supf.reshape(2, 2, SY, C, H, W)
        .transpose(0, 2, 1, 3, 4, 5)
        .reshape(NPAIR, 2 * C, H, W)
    )
    sup_pad = np.zeros((NPAIR, 2 * C, H + 2, WP), np.float32)
    sup_pad[:, :, 1 : H + 1, 1 : W + 1] = sup_pairs
    tgt_pad = np.zeros((2, 2 * C, H + 2, WP), np.float32)
    tgt_pad[:, :, 1 : H + 1, 1 : W + 1] = tgtf.reshape(2, 2 * C, H, W)

    Wt = weight[:, :C]  # applies to target channels
    Ws = weight[:, C:]  # applies to support channels
    wp = np.zeros((128, 9, 128), np.float32)
    wt = np.zeros((128, 9, 128), np.float32)
    for t in range(9):
        ky, kx = divmod(t, 3)
        wp[:64, t, :64] = Ws[:, :, ky, kx].T
        wp[64:, t, 64:] = Ws[:, :, ky, kx].T
        wt[:64, t, :64] = Wt[:, :, ky, kx].T
        wt[64:, t, 64:] = Wt[:, :, ky, kx].T
    wp = np.ascontiguousarray(wp.reshape(128, 9 * 128))
    wt = np.ascontiguousarray(wt.reshape(128, 9 * 128))
    bias2 = np.concatenate([bias, bias]).reshape(1, 128).astype(np.float32)
    ones = np.ones((1, 512), np.float32)

    in_maps = []
    for i in range(NCORES):
        y0 = i * RB
        in_maps.append(
            {
                "sup": np.ascontiguousarray(sup_pad[:, :, y0 : y0 + HR, :]).reshape(
                    NIMG, C, HR * WP
                ),
                "tgt": np.ascontiguousarray(tgt_pad[:, :, y0 : y0 + HR, :]).reshape(
                    2, 128, HR * WP
                ),
                "wp": wp,
                "wt": wt,
                "bias2": bias2,
                "ones": ones,
            }
        )
    return in_maps


def _assemble(results):
    interaction = np.empty((B, SY, CO, H, W), np.float32)
    for i in range(NCORES):
        # out[p] = [2, CO, RB, W] with p = q*16+s covering batches (2q, 2q+1)
        o = results[i]["out"].reshape(2, SY, 2, CO, RB, W)
        o = o.transpose(0, 2, 1, 3, 4, 5).reshape(B, SY, CO, RB, W)
        interaction[:, :, :, i * RB : (i + 1) * RB, :] = o
    new_target = interaction.mean(axis=1, keepdims=True)
    return new_target, interaction


def run(target, support, weight, bias, trace=False, trace_kwargs=None):
    in_maps = _prep_inputs(target, support, weight, bias)
    nc = _build_program()
    kw = {}
    if trace:
        kw = dict(trace=True, **(trace_kwargs or {}))
    res = run_bass_kernel_spmd(nc, in_maps, core_ids=list(range(NCORES)), **kw)
    new_target, interaction = _assemble(res.results)
    return (new_target, interaction), res


def kernel(target, support, weight, bias):
    (new_target, interaction), _ = run(target, support, weight, bias)
    return new_target, interaction
